# revision 5
# baseline (speedup 1.0000x reference)
# Trainium2 Bass kernel for CubeDiagonalAttention.
#
# reference math:
#   z = x @ W.T                         [B, N, 3]
#   s = sign(z)                         (+-1 a.s.)
#   hamming[i,j] = sum_k (s_i,k != s_j,k)
#   bias[i,j] = diag_weights[hamming[i,j]]
#
# Kernel identity (exact): with c_i the 3-bit sign code of row i and
# chi_S(c) = prod_{k in S} s_k the 8 cube characters,
#   bias[i,j] = sum_S (lam_S / 8) chi_S(c_i) chi_S(c_j)
# where lam_S = sum_e diag_weights[popcount(e)] * (-1)^{popcount(S & e)}
# is the eigenvalue of the distance-weight matrix on the hypercube.
# So bias = (Lam * F_q)^T-style K=8 matmul of +-1 character features.
# chi values are +-1 (exact in bf16); for the given diag_weights lam/8
# is exact in bf16 and PSUM f32 accumulation of 8 exact terms is exact,
# so the kernel output matches the reference bit-for-bit given equal
# signs of z (margin: min |z| ~ 2e-5 >> f32 matmul rounding ~1e-6).
#
# Sharding (8 cores): core c -> batch b = c // 2, query-half h = c % 2.
# Each core receives x[b] rolled by -h*2048 rows, computes signs for all
# 4096 rows (keys), uses rows 0:2048 as queries, and emits a [2048, 4096]
# row-block whose columns the host un-rolls.

import sys

import numpy as np

P = 128
B = 4
N = 4096
D = 1024
NQ = 2048
CC = 512  # output column chunk (one PSUM bank of f32)


def _import_concourse():
    try:
        import concourse.bass  # noqa: F401
    except ImportError:
        for p in ("/opt/trn_rl_repo", "/root/.axon_site/_ro/trn_rl_repo"):
            if p not in sys.path:
                sys.path.insert(0, p)
        import concourse.bass  # noqa: F401


def build_program(n=N, d=D, nq=NQ):
    """Emit the SPMD per-core program. Parameterized so a scaled-down
    version can run under CoreSim."""
    _import_concourse()
    from contextlib import ExitStack

    import concourse.mybir as mybir
    import concourse.tile as tile
    from concourse import bacc
    from concourse.masks import make_identity

    f32 = mybir.dt.float32
    bf16 = mybir.dt.bfloat16

    nt = n // P  # key row tiles
    ndc = d // P  # contraction chunks
    nqt = nq // P  # query row tiles
    ncc = n // CC  # output column chunks

    nc = bacc.Bacc()
    xb = nc.declare_dram_parameter("xb", [n, d], f32, isOutput=False)
    wt = nc.declare_dram_parameter("wt", [d, 3], f32, isOutput=False)
    lam = nc.declare_dram_parameter("lam", [8, 1], f32, isOutput=False)
    out = nc.declare_dram_parameter("out", [nq, n], f32, isOutput=True)

    with tile.TileContext(nc) as tc, ExitStack() as ctx:
        const = ctx.enter_context(tc.tile_pool(name="const", bufs=1))
        ident = const.tile([P, P], f32, name="ident")
        make_identity(nc, ident)
        wt_sb = const.tile([P, ndc, 3], f32, name="wt_sb")
        nc.sync.dma_start(out=wt_sb, in_=wt.rearrange("(c p) k -> p c k", p=P))
        lam_sb = const.tile([8, 1], f32, name="lam_sb")
        nc.sync.dma_start(out=lam_sb, in_=lam[:, :])

        # character matrices, bf16: FT[cc] = chi rows for key columns,
        # UFT[rt] = (lam/8)-weighted chi for query rows
        ft = [const.tile([8, CC], bf16, name=f"ft{i}") for i in range(ncc)]
        uft = [const.tile([8, P], bf16, name=f"uft{i}") for i in range(nqt)]

        xpool = ctx.enter_context(tc.tile_pool(name="xpool", bufs=3))
        xtpool = ctx.enter_context(tc.tile_pool(name="xtpool", bufs=10))
        fpool = ctx.enter_context(tc.tile_pool(name="fpool", bufs=4))
        ppool = ctx.enter_context(tc.tile_pool(name="ppool", bufs=3, space="PSUM"))
        zpool = ctx.enter_context(tc.tile_pool(name="zpool", bufs=2, space="PSUM"))

        # ---- phase 1: z = x @ W.T, signs, characters ----
        for t in range(nt):
            xtile = xpool.tile([P, d], f32, name="xtile")
            nc.sync.dma_start(out=xtile, in_=xb[t * P : (t + 1) * P, :])

            xts = []
            for dc in range(ndc):
                tp = ppool.tile([P, P], f32, name="tp", tag="tp")
                nc.tensor.transpose(tp, xtile[:, dc * P : (dc + 1) * P], ident)
                xt = xtpool.tile([P, P], f32, name="xt", tag="xt")
                nc.any.tensor_copy(xt, tp)
                xts.append(xt)
            zp = zpool.tile([P, 3], f32, name="zp", tag="zp")
            for dc in range(ndc):
                nc.tensor.matmul(
                    zp,
                    lhsT=xts[dc],
                    rhs=wt_sb[:, dc, :],
                    start=(dc == 0),
                    stop=(dc == ndc - 1),
                )

            ftile = fpool.tile([P, 8], f32, name="ftile", tag="ftile")
            nc.gpsimd.memset(ftile[:, 0:1], 1.0)
            nc.scalar.sign(ftile[:, 1:4], zp)
            nc.vector.tensor_mul(ftile[:, 4:5], ftile[:, 1:2], ftile[:, 2:3])
            nc.vector.tensor_mul(ftile[:, 5:6], ftile[:, 1:2], ftile[:, 3:4])
            nc.vector.tensor_mul(ftile[:, 6:7], ftile[:, 2:3], ftile[:, 3:4])
            nc.vector.tensor_mul(ftile[:, 7:8], ftile[:, 4:5], ftile[:, 3:4])

            tf = ppool.tile([P, P], f32, name="tf", tag="tp")
            nc.tensor.transpose(tf[0:8, :], ftile, ident)
            cc, off = divmod(t * P, CC)
            nc.vector.tensor_copy(ft[cc][:, off : off + P], tf[0:8, :])
            if t < nqt:
                nc.vector.tensor_scalar_mul(uft[t], tf[0:8, :], lam_sb)

        # ---- phase 3: bias row-block = (lam*F_q)^T . F_k, K=8 matmul ----
        opool = ctx.enter_context(tc.tile_pool(name="opool", bufs=3))
        opsum = ctx.enter_context(tc.tile_pool(name="opsum", bufs=3, space="PSUM"))
        for rt in range(nqt):
            osb = opool.tile([P, n], f32, name="osb", tag="osb")
            for cc in range(ncc):
                pot = opsum.tile([P, CC], f32, name="pot", tag="pot")
                nc.tensor.matmul(
                    pot, lhsT=uft[rt], rhs=ft[cc], start=True, stop=True
                )
                nc.any.tensor_copy(osb[:, cc * CC : (cc + 1) * CC], pot)
            nc.sync.dma_start(out=out[rt * P : (rt + 1) * P, :], in_=osb)

    nc.compile()
    return nc


def _lambda_over_8(diag_weights):
    """lam_S / 8 in character order [1, s1, s2, s3, s1s2, s1s3, s2s3, s1s2s3]
    (subset bitmasks [0, 1, 2, 4, 3, 5, 6, 7])."""
    w = np.asarray(diag_weights, dtype=np.float64)
    lam = np.zeros(8)
    for S in range(8):
        lam[S] = sum(
            w[bin(e).count("1")] * (-1) ** bin(S & e).count("1") for e in range(8)
        ) / 8.0
    order = [0b000, 0b001, 0b010, 0b100, 0b011, 0b101, 0b110, 0b111]
    return lam[order].astype(np.float32).reshape(8, 1)


def kernel(x, W, diag_weights):
    _import_concourse()
    from concourse.bass_utils import run_bass_kernel_spmd

    x = np.ascontiguousarray(np.asarray(x, dtype=np.float32))
    W = np.asarray(W, dtype=np.float32)
    assert x.shape == (B, N, D) and W.shape == (3, D)

    wt = np.ascontiguousarray(W.T)  # [D, 3]
    lam = _lambda_over_8(diag_weights)

    in_maps = []
    for c in range(8):
        b, h = divmod(c, 2)
        xb = x[b] if h == 0 else np.ascontiguousarray(np.roll(x[b], -NQ, axis=0))
        in_maps.append({"xb": xb, "wt": wt, "lam": lam})

    nc = build_program()
    res = run_bass_kernel_spmd(nc, in_maps, list(range(8))).results

    out = np.empty((B, N, N), dtype=np.float32)
    for c in range(8):
        b, h = divmod(c, 2)
        o = res[c]["out"]
        if h:
            o = np.roll(o, NQ, axis=1)
        out[b, h * NQ : (h + 1) * NQ, :] = o
    return out
